# revision 1
# baseline (speedup 1.0000x reference)
"""BandSplitEncoder Trainium2 kernel.

x[B,T,2048] is split into 62 bands (widths 4..256); each band is
RMS-normalized (L2 norm * sqrt(d) * gamma) and passed through its own
Linear[d -> 512]; outputs stack to [B,T,62,512].

Strategy: data-parallel over the 2048 B*T tokens across 8 NeuronCores
(256 tokens each). gamma and sqrt(d) fold into W on the host (the norm
scale is linear in x). The per-token inverse norm commutes with the
matmul, so it is applied to the matmul *output* (a per-partition
scalar) instead of pre-scaling x.

PE-array packing: matmul operands must sit at base partitions that are
legal tile positions (K<=32: 0/32/64/96, K<=64: 0/64, else 0), so the
host repacks the feature axis into 25 zero-padded 128-row chunks, each
holding up to four bands in 32-row slots (d=96 shares its chunk with a
d=24 band at slot 96; d=256 spans two full chunks). Bands in the same
chunk run as concurrent row-tiled matmuls in disjoint PE strips.

Per core (all device tensors fp16 except the f32 norm/PSUM math;
fp16 keeps DMA bytes at half of f32 with ~8x less rounding error than
bf16 at this value range): load x shard natural [256,2048] (norm path)
+ repacked transposed x and W shipped as literal SBUF images
[128, 25*cols] (fully contiguous loads); sumsq per band via ACT
square + segmented DVE reduces,
then sqrt -> clamp(1e-12) -> reciprocal; per band a PE matmul
psum[128tok,512] = xT_band.T @ W_band with f32 accumulate, PSUM->SBUF
copy fused with the per-token scale (ACT and DVE alternate by band),
~8 bands per ~1MB output DMA. The kernel is HBM-bound: ~22.2MB moved
per core at the ~358GB/s per-core limit. b is added on the host (it
broadcasts over tokens).
"""

import numpy as np

import concourse.bacc as bacc
import concourse.tile as tile
from concourse import mybir
from concourse.bass_utils import run_bass_kernel_spmd

# ---------------------------------------------------------------- problem dims
DIM_INPUTS = (4,) * 24 + (8,) * 12 + (24,) * 8 + (48,) * 8 + (96,) * 8 + (256,) * 2
N_BANDS = len(DIM_INPUTS)  # 62
F_TOTAL = sum(DIM_INPUTS)  # 2048
DIM = 512
B, T = 4, 512
BT = B * T  # 2048 tokens
N_CORES = 8
TOK = BT // N_CORES  # 256 tokens per core
N_TILES = TOK // 128  # 2 token tiles per core
EPS = 1e-12

OFFSETS = []
_off = 0
for _d in DIM_INPUTS:
    OFFSETS.append(_off)
    _off += _d

# d-groups for segmented sumsq reduces: (first_band, n_bands, d, col0)
D_GROUPS = []
_i = 0
while _i < N_BANDS:
    d = DIM_INPUTS[_i]
    j = _i
    while j < N_BANDS and DIM_INPUTS[j] == d:
        j += 1
    D_GROUPS.append((_i, j - _i, d, OFFSETS[_i]))
    _i = j

# ------------------------------------------------- packed PE feature layout
# PLACEMENT[band] = list of (chunk, slot, nrows, src_row) matmul segments.
# Slots obey the PE tile-position rule for the segment's K.
PLACEMENT = [None] * N_BANDS
_chunk = 0
for i in range(0, 24, 4):  # d=4: four per chunk
    for j in range(4):
        PLACEMENT[i + j] = [(_chunk, 32 * j, 4, OFFSETS[i + j])]
    _chunk += 1
for i in range(24, 36, 4):  # d=8: four per chunk
    for j in range(4):
        PLACEMENT[i + j] = [(_chunk, 32 * j, 8, OFFSETS[i + j])]
    _chunk += 1
# d=96 + d=24 chunks come before d=48 so chunk load order matches band
# consumption order (d=24 bands 36-43 are consumed before d=48 bands)
for k in range(8):  # d=96 at slot 0, sharing with d=24 at slot 96
    PLACEMENT[52 + k] = [(_chunk, 0, 96, OFFSETS[52 + k])]
    PLACEMENT[36 + k] = [(_chunk, 96, 24, OFFSETS[36 + k])]
    _chunk += 1
for i in range(44, 52, 2):  # d=48: two per chunk (slots 0, 64)
    for j in range(2):
        PLACEMENT[i + j] = [(_chunk, 64 * j, 48, OFFSETS[i + j])]
    _chunk += 1
for k in range(2):  # d=256: two full chunks, accumulated
    PLACEMENT[60 + k] = [
        (_chunk, 0, 128, OFFSETS[60 + k]),
        (_chunk + 1, 0, 128, OFFSETS[60 + k] + 128),
    ]
    _chunk += 2
N_CHUNKS = _chunk  # 25
F_PACK = N_CHUNKS * 128  # 3200

# packed row -> source feature row (or -1 for zero padding)
ROW_MAP = np.full((F_PACK,), -1, dtype=np.int64)
for _b in range(N_BANDS):
    for _c, _slot, _n, _src in PLACEMENT[_b]:
        ROW_MAP[_c * 128 + _slot : _c * 128 + _slot + _n] = np.arange(_src, _src + _n)

# small first/last groups shorten the out-DMA ramp and tail
BAND_GROUPS = [[0, 1]] + [
    list(range(g, min(g + 8, N_BANDS))) for g in range(2, N_BANDS - 4, 8)
] + [[58, 59], [60, 61]]

# const (xt/wg) SBUF tiles are split into groups of chunks so matmuls can
# start as soon as the first slice lands
CONST_SIZES = [3, 8, 14]
assert sum(CONST_SIZES) == N_CHUNKS
CONST_STARTS = [sum(CONST_SIZES[:g]) for g in range(len(CONST_SIZES))]
CHUNK_TO_GROUP = []
for _g, _n in enumerate(CONST_SIZES):
    CHUNK_TO_GROUP += [(_g, _k) for _k in range(_n)]

NORM_SLICES = [(0, N_BANDS, 0, F_TOTAL, D_GROUPS)]

_CACHE = {}


def _build_program():
    nc = bacc.Bacc("TRN2", target_bir_lowering=False, debug=False, num_devices=N_CORES)
    f32 = mybir.dt.float32
    AF = mybir.ActivationFunctionType
    AX = mybir.AxisListType

    f16 = mybir.dt.float16
    xn_ap = nc.dram_tensor("xn", [TOK, F_TOTAL], f16, kind="ExternalInput").ap()
    xt_ap = nc.dram_tensor("xt", [128, N_CHUNKS * TOK], f16, kind="ExternalInput").ap()
    wg_ap = nc.dram_tensor("wg", [128, N_CHUNKS * DIM], f16, kind="ExternalInput").ap()
    out_ap = nc.dram_tensor("out", [TOK, N_BANDS * DIM], f16, kind="ExternalOutput").ap()

    with tile.TileContext(nc) as tc:
        with (
            tc.tile_pool(name="const", bufs=1) as const_pool,
            tc.tile_pool(name="xn", bufs=2) as xn_pool,
            tc.tile_pool(name="xsq", bufs=2) as xsq_pool,
            tc.tile_pool(name="norm", bufs=4) as norm_pool,
            tc.tile_pool(name="inv", bufs=2) as inv_pool,
            tc.tile_pool(name="outb", bufs=10) as out_pool,
            tc.tile_pool(name="psum", bufs=8, space="PSUM") as psum_pool,
        ):
            # stationary activations (packed+transposed) and packed folded
            # weights, split into chunk groups so matmuls start early.
            # CHUNK_TO_GROUP maps chunk c to (group tile, index within
            # group); within a group tile, chunk idx i occupies
            # free cols [i*TOK,(i+1)*TOK) (XT) / [i*DIM,(i+1)*DIM) (WG).
            XTg, WGg = [], []
            for g, ncg in enumerate(CONST_SIZES):
                cs = CONST_STARTS[g]
                XTt = const_pool.tile([128, ncg * TOK], f16, name=f"xtg{g}")
                nc.gpsimd.dma_start(
                    XTt[:], xt_ap[:, cs * TOK : (cs + ncg) * TOK]
                )
                WGt = const_pool.tile([128, ncg * DIM], f16, name=f"wgg{g}")
                nc.gpsimd.dma_start(
                    WGt[:], wg_ap[:, cs * DIM : (cs + ncg) * DIM]
                )
                XTg.append(XTt)
                WGg.append(WGt)

            # ---- norm path for both token tiles up front, in two column
            # slices so early bands' copies unblock sooner:
            # inv[tok, band] = 1/max(||x_band||, eps)
            INVs = [[None] * N_BANDS for _ in range(N_TILES)]
            for b0s, nbs, col0s, ncols, dgs in NORM_SLICES:
                for t in range(N_TILES):
                    XN = xn_pool.tile([128, ncols], f16, name=f"xn{t}_{b0s}")
                    nc.sync.dma_start(
                        XN[:], xn_ap[t * 128 : (t + 1) * 128, col0s : col0s + ncols]
                    )
                    XSQ = xsq_pool.tile([128, ncols], f32, name=f"xsq{t}_{b0s}")
                    nc.scalar.activation(XSQ[:], XN[:], AF.Square)
                    SSQ = norm_pool.tile([128, nbs], f32, name=f"ssq{t}_{b0s}")
                    for b0, nb, d, col0 in dgs:
                        nc.vector.reduce_sum(
                            SSQ[:, b0 - b0s : b0 - b0s + nb],
                            XSQ[:, col0 - col0s : col0 - col0s + nb * d].rearrange(
                                "p (n d) -> p n d", d=d
                            ),
                            axis=AX.X,
                        )
                    NRM = norm_pool.tile([128, nbs], f32, name=f"nrm{t}_{b0s}")
                    nc.scalar.activation(NRM[:], SSQ[:], AF.Sqrt)
                    nc.vector.tensor_scalar_max(NRM[:], NRM[:], EPS)
                    INV = inv_pool.tile([128, nbs], f32, name=f"inv{t}_{b0s}")
                    nc.vector.reciprocal(INV[:], NRM[:])
                    for b in range(b0s, b0s + nbs):
                        INVs[t][b] = INV[:, b - b0s : b - b0s + 1]

            # ---- per-band matmul + scaled copy + grouped DMA out,
            # token tiles interleaved per group to keep the out-DMA fed
            for group in BAND_GROUPS:
                for t in range(N_TILES):
                    OUT = out_pool.tile([128, len(group) * DIM], f16)
                    for j, b_i in enumerate(group):
                        ps = psum_pool.tile([128, DIM], f32, space="PSUM")
                        segs = PLACEMENT[b_i]
                        for k, (c, slot, n, _src) in enumerate(segs):
                            g, i = CHUNK_TO_GROUP[c]
                            nc.tensor.matmul(
                                ps[:],
                                XTg[g][slot : slot + n, i * TOK + t * 128 : i * TOK + (t + 1) * 128],
                                WGg[g][slot : slot + n, i * DIM : (i + 1) * DIM],
                                start=(k == 0),
                                stop=(k == len(segs) - 1),
                                tile_position=(slot, 0),
                            )
                        dst = OUT[:, j * DIM : (j + 1) * DIM]
                        if b_i % 2 == 0:
                            nc.scalar.activation(
                                dst, ps[:], AF.Copy, scale=INVs[t][b_i]
                            )
                        else:
                            nc.vector.tensor_scalar_mul(dst, ps[:], INVs[t][b_i])
                    g0 = group[0]
                    nc.sync.dma_start(
                        out_ap[
                            t * 128 : (t + 1) * 128,
                            g0 * DIM : (g0 + len(group)) * DIM,
                        ],
                        OUT[:],
                    )

    nc.compile()
    return nc


def _get_program():
    if "nc" not in _CACHE:
        _CACHE["nc"] = _build_program()
    return _CACHE["nc"]


def _run(x, gamma, W, b, trace=False, trace_kwargs=None):
    nc = _get_program()

    xf = np.ascontiguousarray(np.asarray(x, dtype=np.float32).reshape(BT, F_TOTAL))
    gamma = np.asarray(gamma, dtype=np.float32)
    W = np.asarray(W, dtype=np.float32)
    b = np.asarray(b, dtype=np.float32)

    # fold gamma and the sqrt(d) norm scale into W rows, then repack
    scale = np.empty((F_TOTAL,), dtype=np.float32)
    for b_i, d in enumerate(DIM_INPUTS):
        scale[OFFSETS[b_i] : OFFSETS[b_i] + d] = np.float32(np.sqrt(d))
    wg = (gamma * scale)[:, None] * W
    valid = ROW_MAP >= 0
    wgp = np.zeros((F_PACK, DIM), dtype=np.float32)
    wgp[valid] = wg[ROW_MAP[valid]]
    # SBUF image: [128 partitions, chunk-major free axis]
    wgp = np.ascontiguousarray(
        wgp.astype(np.float16).reshape(N_CHUNKS, 128, DIM).transpose(1, 0, 2)
    ).reshape(128, N_CHUNKS * DIM)

    in_maps = []
    for i in range(N_CORES):
        shard = np.ascontiguousarray(xf[i * TOK : (i + 1) * TOK])
        xtp = np.zeros((F_PACK, TOK), dtype=np.float32)
        xtp[valid] = shard.T[ROW_MAP[valid]]
        xtp = np.ascontiguousarray(
            xtp.astype(np.float16).reshape(N_CHUNKS, 128, TOK).transpose(1, 0, 2)
        ).reshape(128, N_CHUNKS * TOK)
        in_maps.append({"xn": shard.astype(np.float16), "xt": xtp, "wg": wgp})

    kw = {}
    if trace:
        kw = {"trace": True, "trace_kwargs": trace_kwargs or {}}
    res = run_bass_kernel_spmd(nc, in_maps, core_ids=list(range(N_CORES)), **kw)

    out = np.empty((BT, N_BANDS, DIM), dtype=np.float32)
    for i in range(N_CORES):
        out[i * TOK : (i + 1) * TOK] = res.results[i]["out"].astype(np.float32).reshape(TOK, N_BANDS, DIM)
    out = out.reshape(B, T, N_BANDS, DIM)
    out += b[None, None, :, :]
    return out, res


def kernel(x, gamma, W, b):
    out, _ = _run(x, gamma, W, b)
    return out

